# revision 6
# baseline (speedup 1.0000x reference)
"""MoE gate routing kernel (DeepSeek-V2-style group-limited top-k) for 8x TRN2 NeuronCores.

Problem: nn_MoEGate_13907104105110
  hidden_states [32768, 5120] fp32, gate weight [160, 5120] fp32
  logits = x @ W.T ; scores = softmax(logits)
  group-limited greedy top-k: 8 groups of 20 experts, keep top-3 groups by
  group max score, then top-6 scores of the kept groups, scaled by 16.0.
  Output: [32768, 6] fp32 (top-6 weights, descending).

Sharding: data-parallel over tokens; 4096 tokens per core, W replicated.

Per-core pipeline (per 128-token tile):
  DMA x tile [128, 5120] fp32 (natural layout)
  -> PE transpose per 128-hidden chunk -> PSUM -> copy to SBUF (casts per mode)
  -> PE matmul accumulating logits [128 tok, E] in PSUM
       mode fp32:   1 matmul/chunk, fp32 (4 cyc/row)
       mode fp32r:  1 matmul/chunk, fp32r tf32-like (1 cyc/row at N>=256, padded)
       mode bf16x3: 3 matmuls/chunk, bf16 hi/lo split of both x and W
                    (full fp32-grade accuracy, 1 cyc/row)
  -> softmax via reduce_max(negate) + ACT Exp(bias=-max, accum_out=sum)
  -> group max (reduce over [128, 8, 20]) -> top-8 (vector.max) -> 3rd value
     as group threshold -> mask groups -> top-8 of masked -> first 6 out.
"""

import sys

if "/opt/trn_rl_repo" not in sys.path:
    sys.path.insert(0, "/opt/trn_rl_repo")

from contextlib import ExitStack

import ml_dtypes
import numpy as np

import concourse.bass as bass
import concourse.mybir as mybir
from concourse import bacc
from concourse import tile
from concourse.bass_utils import run_bass_kernel_spmd
from concourse.masks import make_identity

TOKENS = 32768
HIDDEN = 5120
NEXP = 160
EPAD = 256  # fp32r needs moving free dim >= 256 for full rate
TOPK = 6
NGROUP = 8
EPG = NEXP // NGROUP  # 20 experts per group
TOPK_GROUP = 3
SCALE = 16.0
NCORES = 8
TPC = TOKENS // NCORES  # 4096 tokens per core
PT = 128  # tokens per tile
KC = HIDDEN // 128  # 40 contraction chunks

F32 = mybir.dt.float32
F32R = mybir.dt.float32r
BF16 = mybir.dt.bfloat16

MM_MODE = "bf16x3"


def build_nc(tokens_per_core: int = TPC, mm_mode: str = MM_MODE) -> bass.Bass:
    nt = tokens_per_core // PT
    nc = bacc.Bacc("TRN2", target_bir_lowering=False, debug=False)
    x_dram = nc.dram_tensor("x", [tokens_per_core, HIDDEN], F32, kind="ExternalInput")
    if mm_mode == "fp32":
        w_shape, w_dt, ne = [128, KC, NEXP], F32, NEXP
    elif mm_mode == "fp32r":
        w_shape, w_dt, ne = [128, KC, EPAD], F32R, EPAD
    elif mm_mode == "bf16x3":
        w_shape, w_dt, ne = [128, KC, 2, NEXP], BF16, NEXP
    else:
        raise ValueError(mm_mode)
    # w pre-arranged on host: hidden chunk on partitions (see prep_w)
    w_dram = nc.dram_tensor("w", w_shape, w_dt, kind="ExternalInput")
    out_dram = nc.dram_tensor("out", [tokens_per_core, TOPK], F32, kind="ExternalOutput")

    xt_dt = {"fp32": F32, "fp32r": F32R, "bf16x3": BF16}[mm_mode]

    with tile.TileContext(nc) as tc, ExitStack() as ctx:
        const_pool = ctx.enter_context(tc.tile_pool(name="const", bufs=1))
        x_pool = ctx.enter_context(tc.tile_pool(name="x", bufs=2))
        xt_pool = ctx.enter_context(tc.tile_pool(name="xt", bufs=2))
        ps_tr_pool = ctx.enter_context(tc.tile_pool(name="ps_tr", bufs=4, space="PSUM"))
        ps_lg_pool = ctx.enter_context(tc.tile_pool(name="ps_lg", bufs=2, space="PSUM"))
        rt_pool = ctx.enter_context(tc.tile_pool(name="rt", bufs=2))
        st_pool = ctx.enter_context(tc.tile_pool(name="st", bufs=2))

        w_sb = const_pool.tile(w_shape, w_dt)
        nc.sync.dma_start(w_sb[:], w_dram[:])
        ident = const_pool.tile([128, 128], F32)
        make_identity(nc, ident[:])

        for t in range(nt):
            x_sb = x_pool.tile([128, HIDDEN], F32)
            nc.sync.dma_start(x_sb[:], x_dram[t * PT : (t + 1) * PT, :])

            # transpose phase: x tile -> xT [128 hidden, KC, 128 tokens]
            xt_hi = xt_pool.tile([128, KC, 128], xt_dt, tag="xt_hi")
            if mm_mode == "bf16x3":
                xt_lo = xt_pool.tile([128, KC, 128], BF16, tag="xt_lo")
            for k in range(KC):
                xt_ps = ps_tr_pool.tile([128, 128], F32)
                nc.tensor.transpose(xt_ps[:], x_sb[:, k * 128 : (k + 1) * 128], ident[:])
                if mm_mode == "bf16x3":
                    # hi = bf16(xT); lo = bf16(xT - hi)
                    nc.scalar.copy(xt_hi[:, k, :], xt_ps[:])
                    nc.vector.tensor_sub(xt_lo[:, k, :], xt_ps[:], xt_hi[:, k, :])
                else:
                    if k % 2 == 0:
                        nc.vector.tensor_copy(xt_hi[:, k, :], xt_ps[:])
                    else:
                        nc.scalar.copy(xt_hi[:, k, :], xt_ps[:])

            # matmul phase: logits[tok, e] += xT_k.T @ W_k
            lg_ps = ps_lg_pool.tile([128, ne], F32)
            for k in range(KC):
                if mm_mode == "bf16x3":
                    nc.tensor.matmul(lg_ps[:], xt_hi[:, k, :], w_sb[:, k, 0, :],
                                     start=(k == 0), stop=False)
                    nc.tensor.matmul(lg_ps[:], xt_hi[:, k, :], w_sb[:, k, 1, :],
                                     start=False, stop=False)
                    nc.tensor.matmul(lg_ps[:], xt_lo[:, k, :], w_sb[:, k, 0, :],
                                     start=False, stop=(k == KC - 1))
                else:
                    nc.tensor.matmul(lg_ps[:], xt_hi[:, k, :], w_sb[:, k, :],
                                     start=(k == 0), stop=(k == KC - 1))

            # routing phase
            logits = lg_ps[:, :NEXP]
            negmax = rt_pool.tile([128, 1], F32, tag="negmax")
            nc.vector.tensor_reduce(
                negmax[:], logits, axis=mybir.AxisListType.X, op=mybir.AluOpType.max, negate=True
            )
            escore = st_pool.tile([128, NEXP], F32, tag="escore")
            ssum = rt_pool.tile([128, 1], F32, tag="ssum")
            nc.scalar.activation(
                escore[:], logits, mybir.ActivationFunctionType.Exp,
                bias=negmax[:], scale=1.0, accum_out=ssum[:],
            )
            rec = rt_pool.tile([128, 1], F32, tag="rec")
            nc.vector.reciprocal(rec[:], ssum[:])
            scores = st_pool.tile([128, NEXP], F32, tag="scores")
            # scores = escore * rec * SCALE (scaling is monotone; threshold works on same tensor)
            nc.vector.tensor_scalar(
                scores[:], escore[:], rec[:], SCALE,
                op0=mybir.AluOpType.mult, op1=mybir.AluOpType.mult,
            )
            gs = rt_pool.tile([128, NGROUP], F32, tag="gs")
            nc.vector.tensor_reduce(
                gs[:], scores[:].rearrange("p (g e) -> p g e", e=EPG),
                axis=mybir.AxisListType.X, op=mybir.AluOpType.max,
            )
            g8 = rt_pool.tile([128, 8], F32, tag="g8")
            nc.vector.max(out=g8[:], in_=gs[:])
            gmask = rt_pool.tile([128, NGROUP], F32, tag="gmask")
            nc.vector.tensor_scalar(
                gmask[:], gs[:], g8[:, TOPK_GROUP - 1 : TOPK_GROUP], None,
                op0=mybir.AluOpType.is_ge,
            )
            masked = st_pool.tile([128, NEXP], F32, tag="masked")
            nc.vector.tensor_tensor(
                masked[:].rearrange("p (g e) -> p g e", e=EPG),
                scores[:].rearrange("p (g e) -> p g e", e=EPG),
                gmask[:].to_broadcast([128, NGROUP, EPG]),
                op=mybir.AluOpType.mult,
            )
            top8 = rt_pool.tile([128, 8], F32, tag="top8")
            nc.vector.max(out=top8[:], in_=masked[:])
            nc.sync.dma_start(out_dram[t * PT : (t + 1) * PT, :], top8[:, :TOPK])

    nc.compile()
    return nc


def _round_fp32r(a: np.ndarray) -> np.ndarray:
    """Round-to-nearest-even to 12-bit significand (tf32-like fp32r)."""
    bits = a.astype(np.float32).view(np.uint32).astype(np.uint64)
    lsb = (bits >> 12) & 1
    rounded = (bits + 0x7FF + lsb) & 0xFFFFF000
    return rounded.astype(np.uint32).view(np.float32)


def prep_w(kernel_w: np.ndarray, mm_mode: str = MM_MODE) -> np.ndarray:
    w = np.asarray(kernel_w, dtype=np.float32)
    if mm_mode == "fp32":
        # [NEXP, HIDDEN] -> [HIDDEN, NEXP] -> [KC, 128, NEXP] -> [128, KC, NEXP]
        return np.ascontiguousarray(w.T.reshape(KC, 128, NEXP).transpose(1, 0, 2))
    if mm_mode == "fp32r":
        wpad = np.zeros((EPAD, HIDDEN), np.float32)
        wpad[:NEXP] = _round_fp32r(w)
        return np.ascontiguousarray(wpad.T.reshape(KC, 128, EPAD).transpose(1, 0, 2))
    if mm_mode == "bf16x3":
        whi = w.astype(ml_dtypes.bfloat16)
        wlo = (w - whi.astype(np.float32)).astype(ml_dtypes.bfloat16)
        # [2, NEXP, HIDDEN] -> [HIDDEN, 2, NEXP] -> [KC, 128, 2, NEXP] -> [128, KC, 2, NEXP]
        wb = np.stack([whi, wlo])  # [2, NEXP, HIDDEN]
        return np.ascontiguousarray(
            wb.transpose(2, 0, 1).reshape(KC, 128, 2, NEXP).transpose(1, 0, 2, 3)
        )
    raise ValueError(mm_mode)


def run(hidden_states: np.ndarray, kernel_w: np.ndarray, mm_mode: str = MM_MODE, **spmd_kwargs):
    x = np.ascontiguousarray(hidden_states, dtype=np.float32)
    w_arr = prep_w(kernel_w, mm_mode)
    nc = build_nc(TPC, mm_mode=mm_mode)
    in_maps = [
        {"x": x[i * TPC : (i + 1) * TPC], "w": w_arr} for i in range(NCORES)
    ]
    res = run_bass_kernel_spmd(nc, in_maps, list(range(NCORES)), **spmd_kwargs)
    out = np.concatenate([res.results[i]["out"] for i in range(NCORES)], axis=0)
    return out, res


def kernel(hidden_states: np.ndarray, kernel: np.ndarray) -> np.ndarray:
    return run(hidden_states, kernel)[0]
